# revision 4
# baseline (speedup 1.0000x reference)
"""Multi-head attention on 8 TRN2 NeuronCores (data/head-parallel).

Problem: B=4 H=16 S=2048 D=64 fp32 attention, out = softmax(Q K^T / sqrt(D)) V.
B*H = 64 (batch, head) pairs are sharded 8-per-core; each core runs the same
NEFF over its own 8 heads, no collectives.

The baseline bf16 kernel was jointly PE-bound and ACT-bound (~260us busy
each).  This version attacks both:

  - Q K^T exploits PE row tiling: with contraction d=64, two k-tiles are
    computed CONCURRENTLY in the two 64-row halves of the PE array
    (tile_position (0,0) / (64,0)).  Q^T is duplicated into partitions
    64..127, even k-tiles live in rows 0..63, odd k-tiles in rows 64..127.
    Both matmuls of a pair run overlapped (measured ~2x on row tiling),
    cutting QK^T PE time roughly in half at full bf16 accuracy.
  - exp is split across engines: even k-tiles on ACT (hw exp), odd k-tiles
    on the otherwise-idle DVE as a Schraudolph bit-hack: E = bitcast_bf16(
    int16(A*s + B)), one tensor_scalar per half-tile.  Multiplicative
    sawtooth error ~1.8% rms on those tiles; softmax renormalization
    cancels the mean.  Each exp is issued as two [128,512] instructions so
    the first half starts as soon as the first matmul of the pair lands.
  - P@V stays bf16 (fp8 probabilities/values fail the 2e-2 error budget).
  - The output [d,q]->[q,d] transpose runs on the DMA XBAR
    (dma_start_transpose of a bf16 [80,1024] tile), not the PE; PSUM then
    fits score A/B tiles plus a double-buffered P@V accumulator (8 banks).
  - Normalization scales run on GpSimd (SBUF-resident after the XBAR
    transpose), keeping DVE free for exp work.
  - Software pipeline: iteration g runs QK+exp of chunk g interleaved with
    P@V of chunk g-1 on the PE, then the PSUM->SBUF copy of g-1 (ACT) and
    the normalize+store of g-2 (DVE reciprocal, GpSimd scale, Sync DMA).
"""

import math
from contextlib import ExitStack

import ml_dtypes
import numpy as np

import concourse.bass as bass
import concourse.bacc as bacc
import concourse.tile as tile
import concourse.mybir as mybir
from concourse.bass_utils import run_bass_kernel_spmd

B, H, S, D = 4, 16, 2048, 64
N_CORES = 8
HPC = B * H // N_CORES     # heads per core
ST = S // 128              # 16 k-tiles of 128
NP = ST // 2               # 8 row-tiled k-tile pairs
QCHUNK = 1024              # q processed in chunks (PSUM budget)
NQ = S // QCHUNK
NJ = QCHUNK // 128         # 128-q output groups per chunk
DT = mybir.dt

# Schraudolph int16/bf16 exp: E = bitcast_bf16(int16(A*s + B)), trunc-calibrated
SCHRAUD_A = 128.0 / math.log(2.0)                # * scale at runtime
SCHRAUD_B = 127.0 * 128.0 + 0.5 - 0.0430 * 128.0

_BUILT = {}


class _Bacc(bacc.Bacc):
    """Bacc with the move-matmul-waits-to-ldweights pass disabled: keeping
    waits on the matmul (not its LDWEIGHTS) lets the PE queue pull weight
    loads ahead of in-flight matmuls, hiding the LDW cost."""

    def move_matmul_waits_to_ldweights(self):
        pass


def _load_head(nc, stage, qt_d, kt_d, vp_d, h, first):
    qt = stage.tile([128, S], DT.bfloat16, tag="qt")
    kt = stage.tile([128, NP, 128], DT.bfloat16, tag="kt")
    vp = stage.tile([128, ST, 128], DT.bfloat16, tag="vp")
    for j in range(2):
        half = slice(j * (S // 2), (j + 1) * (S // 2))
        jh = slice(j * (NP // 2), (j + 1) * (NP // 2))
        # Cold start: head 0's first halves ride the idle Sync/Scalar HWDGEs.
        keng = nc.sync if (first and j == 0) else nc.gpsimd
        qeng = nc.scalar if (first and j == 0) else nc.gpsimd
        keng.dma_start(out=kt[:, jh, :], in_=kt_d[h][:, jh, :])
        qeng.dma_start(out=qt[:, half], in_=qt_d[h][:, half])
    vp_v = vp_d[h].rearrange("(t p) e -> p t e", p=128)
    for j in range(2):
        sl = slice(8 * j, 8 * j + 8)
        nc.gpsimd.dma_start(out=vp[:, sl, :], in_=vp_v[:, sl, :])
    return qt, kt, vp


def build_graph(scale: float, heads: int = HPC):
    nc = _Bacc("TRN2", target_bir_lowering=False, debug=False,
               num_devices=N_CORES)
    qt_d = nc.dram_tensor("QT", [heads, 128, S], DT.bfloat16,
                          kind="ExternalInput").ap()
    kt_d = nc.dram_tensor("KT", [heads, 128, NP, 128], DT.bfloat16,
                          kind="ExternalInput").ap()
    vp_d = nc.dram_tensor("VP", [heads, S, 128], DT.bfloat16,
                          kind="ExternalInput").ap()
    o_d = nc.dram_tensor("out", [heads, S, D], DT.float32,
                         kind="ExternalOutput").ap()

    a_s = float(scale) * SCHRAUD_A

    with tile.TileContext(nc) as tc, ExitStack() as ctx:
        stage = ctx.enter_context(tc.tile_pool(name="stage", bufs=3))
        epool = ctx.enter_context(tc.tile_pool(name="epool", bufs=2))
        spool = ctx.enter_context(tc.tile_pool(name="spool", bufs=2))
        trp = ctx.enter_context(tc.tile_pool(name="trp", bufs=2))
        outp = ctx.enter_context(tc.tile_pool(name="outp", bufs=2))
        recp = ctx.enter_context(tc.tile_pool(name="recp", bufs=2))
        ps_sa = ctx.enter_context(tc.tile_pool(name="ps_sa", bufs=1, space="PSUM"))
        ps_sb = ctx.enter_context(tc.tile_pool(name="ps_sb", bufs=1, space="PSUM"))
        ps_ot = ctx.enter_context(tc.tile_pool(name="ps_ot", bufs=2, space="PSUM"))

        gs = [(h, c) for h in range(heads) for c in range(NQ)]
        head_tiles = {}
        state = {}   # iteration -> dict(ets, vp, ot, otr, h, c)

        head_tiles[0] = _load_head(nc, stage, qt_d, kt_d, vp_d, 0, True)

        for i in range(len(gs) + 2):
            cur = gs[i] if i < len(gs) else None
            if cur is not None:
                h, c = cur
                if c == NQ - 1 and h + 1 < heads:
                    head_tiles[h + 1] = _load_head(nc, stage, qt_d, kt_d,
                                                   vp_d, h + 1, False)
                qt, kt, vp = head_tiles[h]
                q0 = c * QCHUNK
                ets = []
                state[i] = {"h": h, "c": c, "vp": vp, "ets": ets}
            prev = state.get(i - 1)
            fin = state.pop(i - 2, None)

            for j in range(NP):
                if cur is not None:
                    sta = ps_sa.tile([128, QCHUNK], DT.float32, tag="sta")
                    stb = ps_sb.tile([128, QCHUNK], DT.float32, tag="stb")
                    for n in range(2):
                        qsl = slice(q0 + n * 512, q0 + (n + 1) * 512)
                        osl = slice(n * 512, (n + 1) * 512)
                        # Two concurrent matmuls in the PE row halves.
                        nc.tensor.matmul(sta[:, osl], lhsT=kt[0:64, j, :],
                                         rhs=qt[0:64, qsl],
                                         start=True, stop=True)
                        nc.tensor.matmul(stb[:, osl], lhsT=kt[64:128, j, :],
                                         rhs=qt[64:128, qsl],
                                         start=True, stop=True)
                    # even k-tile (rows 0..63) -> ACT exp, halves pipelined
                    et = epool.tile([128, QCHUNK], DT.bfloat16, tag=f"et{2*j}")
                    for n in range(2):
                        osl = slice(n * 512, (n + 1) * 512)
                        nc.scalar.activation(
                            out=et[:, osl], in_=sta[:, osl],
                            func=mybir.ActivationFunctionType.Exp, scale=scale)
                    ets.append(et)
                    # odd k-tile -> DVE Schraudolph
                    eti = epool.tile([128, QCHUNK], DT.int16, tag=f"et{2*j+1}")
                    for n in range(2):
                        osl = slice(n * 512, (n + 1) * 512)
                        nc.vector.tensor_scalar(
                            eti[:, osl], stb[:, osl], a_s, SCHRAUD_B,
                            mybir.AluOpType.mult, mybir.AluOpType.add)
                    ets.append(eti.bitcast(DT.bfloat16))
                if prev is not None:
                    if j == 0:
                        prev["ot"] = ps_ot.tile([128, QCHUNK], DT.float32,
                                                tag="ot", name="ot")
                    for t in (2 * j, 2 * j + 1):
                        for n in range(2):
                            osl = slice(n * 512, (n + 1) * 512)
                            nc.tensor.matmul(
                                prev["ot"][:, osl],
                                lhsT=prev["vp"][:, t, :],
                                rhs=prev["ets"][t][:, osl],
                                start=(t == 0), stop=(t == ST - 1),
                            )

            if prev is not None:
                # PSUM -> SBUF as bf16 (ACT), then [80,1024] -> [1024,80] on
                # the DMA XBAR.  Rows 65..79 are the zero-padded V columns.
                ots = spool.tile([80, QCHUNK], DT.bfloat16, tag="ots")
                nc.scalar.copy(out=ots, in_=prev["ot"][0:80, :])
                otr = trp.tile([128, NJ, 80], DT.bfloat16, tag="otr")
                nc.sync.dma_start_transpose(out=otr, in_=ots)
                prev["otr"] = otr

            if fin is not None:
                otr = fin["otr"]
                rec = recp.tile([128, NJ], DT.float32, tag="rec")
                nc.vector.reciprocal(out=rec, in_=otr[:, :, D])
                outst = outp.tile([128, NJ, D], DT.float32, tag="outst")
                for j in range(NJ):
                    nc.gpsimd.tensor_scalar(
                        outst[:, j, :], otr[:, j, 0:D], rec[:, j:j + 1],
                        None, mybir.AluOpType.mult)
                o_v = o_d[fin["h"], fin["c"] * QCHUNK:(fin["c"] + 1) * QCHUNK, :]
                o_v = o_v.rearrange("(r p) d -> p r d", p=128)
                nc.sync.dma_start(out=o_v, in_=outst)

    nc.compile()
    return nc


def _get_nc(scale: float):
    key = round(float(scale), 9)
    if key not in _BUILT:
        _BUILT[key] = build_graph(float(scale))
    return _BUILT[key]


def shard_inputs(Q, K, V):
    """Host-side prep: shard heads across cores.  Q^T duplicated into both
    64-row halves; K^T split so even k-tiles sit in partitions 0..63 and odd
    k-tiles in 64..127 (PE row tiling).  V gets a ones column (softmax
    denominators fall out of the P@V matmul) and bf16."""
    bf16 = ml_dtypes.bfloat16
    BH = B * H
    qs = np.asarray(Q, dtype=np.float32).reshape(BH, S, D).transpose(0, 2, 1)
    ks = np.asarray(K, dtype=np.float32).reshape(BH, S, D).transpose(0, 2, 1)
    vs = np.asarray(V, dtype=np.float32).reshape(BH, S, D)

    qt = np.empty((BH, 128, S), dtype=bf16)
    qt[:, :D, :] = qs.astype(bf16)
    qt[:, D:, :] = qt[:, :D, :]

    kv = ks.astype(bf16).reshape(BH, D, ST, 128)
    kt = np.empty((BH, 128, NP, 128), dtype=bf16)
    kt[:, :D] = kv[:, :, 0::2, :]
    kt[:, D:] = kv[:, :, 1::2, :]

    vp = np.zeros((BH, S, 128), dtype=bf16)
    vp[:, :, :D] = vs.astype(bf16)
    vp[:, :, D] = np.float32(1.0)

    in_maps = []
    for c in range(N_CORES):
        sl = slice(c * HPC, (c + 1) * HPC)
        in_maps.append({
            "QT": np.ascontiguousarray(qt[sl]),
            "KT": np.ascontiguousarray(kt[sl]),
            "VP": np.ascontiguousarray(vp[sl]),
        })
    return in_maps


def kernel(Q, K, V, d_k, **run_kwargs):
    scale = 1.0 / math.sqrt(float(d_k))
    nc = _get_nc(scale)
    in_maps = shard_inputs(Q, K, V)
    res = run_bass_kernel_spmd(nc, in_maps, core_ids=list(range(N_CORES)),
                               **run_kwargs)
    out = np.concatenate([r["out"] for r in res.results], axis=0)
    out = out.reshape(B, H, S, D).astype(np.float32)
    kernel.last_results = res
    return out
